# revision 20
# baseline (speedup 1.0000x reference)
"""Trainium2 Bass kernel for nn_BaseLineModel (hierarchical sentence->doc model).

v3 pipeline per core (4 docs, 128 sentences, position-major):
  embedding gather (indirect DMA, bf16) -> PE transpose (bf16, 1 cyc/row)
  -> fp8 ebt/tails -> conv as fp8 DoubleRow matmuls (K=256/pass, 8 passes/fc)
  over blocks of 3 positions with hoisted stationaries -> tanh -> fp8 c0
  -> attn0 logits fp8 DoubleRow -> tanh -> exp -> segmented tensor_reduce
  softmax -> s0 (one TT per position).
  x-projection lands in 2 persistent PSUM banks (4 chunks of 8 positions);
  LSTM whh matmuls accumulate onto it (start=False) and steps are woven
  into later conv blocks so the recurrence hides under phase A.
  Final masked sentence attention + sigmoid output.

Data-parallel over docs: core k handles docs 4k..4k+3 end-to-end; host
concatenates the 8 [4,1] outputs. No collectives.
"""
import sys

for _p in ("/opt/trn_rl_repo", "/root/.axon_site/_ro/trn_rl_repo"):
    if _p not in sys.path:
        sys.path.insert(0, _p)

from contextlib import ExitStack

import numpy as np
import ml_dtypes

import concourse.bass as bass
import concourse.tile as tile
from concourse import mybir
from concourse.bass import IndirectOffsetOnAxis
from concourse.bass_utils import run_bass_kernel_spmd
from concourse.masks import make_identity
from concourse.tile import TileContext

from concourse.vector_clock import ScopedClock


class _TC(TileContext):
    """TileContext that limits every instruction to a single sem wait
    (this walrus rejects multiple sync waits on one instruction); extra
    waits are spilled onto preceding same-engine nops."""

    def _commit_instruction(self, inst, lazy_reg_writes: bool = True):
        si = getattr(inst, "sync_info", None)
        if (
            si is not None
            and si.on_wait
            and len(si.on_wait) > 1
            and inst.engine != mybir.EngineType.Unassigned
        ):
            waits = list(si.on_wait)
            inst.sync_info = mybir.SyncInfo(
                on_wait=[waits[-1]], on_update=list(si.on_update or []))
            eng = self.nc.engines[inst.engine]
            for w in waits[:-1]:
                nop = eng.nop().ins
                nop.sync_info = mybir.SyncInfo(on_wait=[w], on_update=[])
        return super()._commit_instruction(inst, lazy_reg_writes)

    def _drain_and_barrier(self, tick_clock, wait_clock):
        carrier = self.nc.sync.nop().ins
        wait_clock.add_sem_waits(
            carrier, ScopedClock({None: tick_clock.global_clock}))
        si = carrier.sync_info
        if si is not None and si.on_wait and len(si.on_wait) > 1:
            waits = list(si.on_wait)
            carrier.sync_info = mybir.SyncInfo(
                on_wait=[waits[0]], on_update=list(si.on_update or []))
            for w in waits[1:]:
                n2 = self.nc.sync.nop().ins
                n2.sync_info = mybir.SyncInfo(on_wait=[w], on_update=[])
        self.nc.sync.drain()
        self.nc.all_engine_barrier()
        assert self.sems is not None
        popped = self.nc._tile_sem_poison_stack.pop()
        assert popped is self._sem_poison
        self.nc.clear_and_free_semaphores(list(self.sems.allocated().values()))
        self.nc.all_engine_barrier()


BF16 = mybir.dt.bfloat16
FP32 = mybir.dt.float32
FP8 = mybir.dt.float8e4
INT32 = mybir.dt.int32
AF = mybir.ActivationFunctionType
OP = mybir.AluOpType
AX = mybir.AxisListType
DR = mybir.MatmulPerfMode.DoubleRow

B, S, L = 32, 64, 128
TOTAL = 1024
V, E, F, W, H = 30000, 300, 256, 5, 256
EP = 304                 # bf16 table row padded to 304 cols
T = L - W + 1            # 124 valid conv positions
NCORES = 8
DPC = B // NCORES        # 4 docs per core
ECH = [128, 128, 44]     # E=300 split into K-chunks
BLK = 4                  # conv positions per stationary-hoisted block
bf16 = ml_dtypes.bfloat16
f8 = ml_dtypes.float8_e4m3
# LSTM gate-chunk order: PyTorch (i,i,f,f,g,g,o,o) -> packed (i,i,f,f,o,o,g,g)
GPERM = [0, 1, 2, 3, 6, 7, 4, 5]


def build_nc(S_eff: int, n_groups: int, group_targets):
    """group_targets[g] = list of 4 entries: (t, d) slot or None (pad),
    shared by all cores."""
    NLOC = n_groups * 4
    S4 = S_eff * DPC

    # interleaved schedule only for the canonical position-major layout
    uniform_pm = (S_eff == n_groups) and all(
        group_targets[g] == [(g, d) for d in range(DPC)]
        for g in range(n_groups))

    nc = bass.Bass()
    # ---- DRAM I/O ----
    d_idx = nc.dram_tensor("idx_t", [128, NLOC], INT32, kind="ExternalInput")
    d_wemb = nc.dram_tensor("wembt", [V, EP], BF16, kind="ExternalInput")
    d_wmain = nc.dram_tensor("wmain", [128, W, 2, 2, 128], FP8, kind="ExternalInput")
    d_wtail = nc.dram_tensor("wtail", [44, 3, 2, 2, 128], FP8, kind="ExternalInput")
    d_bconv = nc.dram_tensor("bconv_t", [128, 2], FP32, kind="ExternalInput")
    d_wa0 = nc.dram_tensor("wa0_t", [128, 2, 2, 128], FP8, kind="ExternalInput")
    d_ba0 = nc.dram_tensor("ba0_t", [128, 2], FP32, kind="ExternalInput")
    d_wih = nc.dram_tensor("wih_t", [128, 2, 8, 128], BF16, kind="ExternalInput")
    d_bihrow = nc.dram_tensor("bihrow", [1, 8, 128], BF16, kind="ExternalInput")
    d_whh = nc.dram_tensor("whh_t", [128, 2, 8, 128], BF16, kind="ExternalInput")
    d_wa1 = nc.dram_tensor("wa1_t", [128, 2, 2, 128], BF16, kind="ExternalInput")
    d_ba1 = nc.dram_tensor("ba1_t", [128, 2], FP32, kind="ExternalInput")
    d_mb1 = nc.dram_tensor("mb1", [1, S4], FP32, kind="ExternalInput")
    d_wo = nc.dram_tensor("wo_t", [128, 2], BF16, kind="ExternalInput")
    d_boh = nc.dram_tensor("bo_half", [1, 1], FP32, kind="ExternalInput")
    d_out = nc.dram_tensor("out", [1, DPC], FP32, kind="ExternalOutput")
    import os as _os
    dbg = bool(_os.environ.get("KDBG"))
    if dbg:
        d_dbg_s0 = nc.dram_tensor("dbg_s0", [128, 2, S_eff, DPC], BF16,
                                  kind="ExternalOutput")
        d_dbg_h = nc.dram_tensor("dbg_h", [128, S_eff, 8], BF16,
                                 kind="ExternalOutput")
        d_dbg_xp = nc.dram_tensor("dbg_xp", [128, 2, 16, 32], FP32,
                                  kind="ExternalOutput")
        d_dbg_c0 = nc.dram_tensor("dbg_c0", [128, 2, 512], FP8,
                                  kind="ExternalOutput")

    n_chunks = (S_eff + 7) // 8

    with _TC(nc) as tc, ExitStack() as ctx:
        consts = ctx.enter_context(tc.tile_pool(name="consts", bufs=1))

        identb = consts.tile([128, 128], BF16)
        make_identity(nc, identb[:, :])
        ones32 = consts.tile([1, 32], BF16)
        nc.vector.memset(ones32[:, :], 1.0)
        idx_sb = consts.tile([128, NLOC], INT32)
        nc.sync.dma_start(out=idx_sb[:, :], in_=d_idx[:, :])
        wmain_sb = consts.tile([128, W, 2, 2, 128], FP8)
        nc.sync.dma_start(out=wmain_sb[:, :, :, :, :], in_=d_wmain[:, :, :, :, :])
        wtail_sb = consts.tile([44, 3, 2, 2, 128], FP8)
        nc.sync.dma_start(out=wtail_sb[:, :, :, :, :], in_=d_wtail[:, :, :, :, :])
        bconv_sb = consts.tile([128, 2], FP32)
        nc.sync.dma_start(out=bconv_sb[:, :], in_=d_bconv[:, :])
        wa0_sb = consts.tile([128, 2, 2, 128], FP8)
        nc.sync.dma_start(out=wa0_sb[:, :, :, :], in_=d_wa0[:, :, :, :])
        ba0_sb = consts.tile([128, 2], FP32)
        nc.sync.dma_start(out=ba0_sb[:, :], in_=d_ba0[:, :])
        wih_sb = consts.tile([128, 2, 8, 128], BF16)
        nc.sync.dma_start(out=wih_sb[:, :, :, :], in_=d_wih[:, :, :, :])
        bihrow_sb = consts.tile([1, 8, 128], BF16)
        nc.sync.dma_start(out=bihrow_sb[:, :, :], in_=d_bihrow[:, :, :])
        whh_sb = consts.tile([128, 2, 8, 128], BF16)
        nc.sync.dma_start(out=whh_sb[:, :, :, :], in_=d_whh[:, :, :, :])
        wa1_sb = consts.tile([128, 2, 2, 128], BF16)
        nc.sync.dma_start(out=wa1_sb[:, :, :, :], in_=d_wa1[:, :, :, :])
        ba1_sb = consts.tile([128, 2], FP32)
        nc.sync.dma_start(out=ba1_sb[:, :], in_=d_ba1[:, :])
        mb1_sb = consts.tile([128, S4], FP32)
        nc.sync.dma_start(
            out=mb1_sb[:, :], in_=d_mb1[:, :].to_broadcast([128, S4]))
        wo_sb = consts.tile([128, 2], BF16)
        nc.sync.dma_start(out=wo_sb[:, :], in_=d_wo[:, :])
        boh_sb = consts.tile([1, 1], FP32)
        nc.sync.dma_start(out=boh_sb[:, :], in_=d_boh[:, :])

        # persistent accumulators
        s0T_sb = consts.tile([128, 2, S_eff, DPC], BF16)
        nc.vector.memset(s0T_sb[:, :, :, :], 0.0)
        s0scr = consts.tile([128, 2, 4], BF16)
        c_sb = consts.tile([128, 8], FP32)
        hsb_sb = consts.tile([128, S_eff, 8], BF16)

        p_xps = ctx.enter_context(
            tc.tile_pool(name="xps", bufs=1, space="PSUM"))
        xps0 = p_xps.tile([128, 16, 32], FP32, tag="xp0")
        xps1 = p_xps.tile([128, 16, 32], FP32, tag="xp1")
        xps = [xps0, xps1]
        # DVE writes do not set PSUM has_written bits: after this memset the
        # first PE write per element overwrites, later ones accumulate --
        # robust against bank-level has_written clears from start/stop
        nc.vector.memset(xps0[:, :, :], 0.0)
        nc.vector.memset(xps1[:, :, :], 0.0)
        p_lst = ctx.enter_context(tc.tile_pool(name="lst", bufs=2))

        # -------- phase helpers (emitted in different orders below) --------
        def emit_xproj_chunk(c):
            lo = 8 * c
            hi = min(lo + 8, S_eff)
            xp = xps[(lo // 16) % 2]
            tsl = slice(lo % 16, lo % 16 + (hi - lo))
            for gt in range(8):
                for kc in range(2):
                    nc.tensor.matmul(
                        out=xp[:, tsl, 4 * gt:4 * gt + 4],
                        lhsT=wih_sb[:, kc, gt, :],
                        rhs=s0T_sb[:, kc, lo:hi, :],
                        start=False, stop=False,
                        skip_group_check=True,
                    )
                nc.tensor.matmul(
                    out=xp[:, tsl, 4 * gt:4 * gt + 4],
                    lhsT=bihrow_sb[:, gt, :],
                    rhs=ones32[:, 0:4 * (hi - lo)],
                    start=False, stop=False,
                    skip_group_check=True,
                )

        def emit_lstm_step(t):
            xp = xps[(t // 16) % 2]
            r = t % 16
            if t > 0:
                for gt in range(8):
                    for kc in range(2):
                        nc.tensor.matmul(
                            out=xp[:, r, 4 * gt:4 * gt + 4],
                            lhsT=whh_sb[:, kc, gt, :],
                            rhs=hsb_sb[:, t - 1, 4 * kc:4 * kc + 4],
                            start=False, stop=(kc == 1),
                            skip_group_check=True,
                        )
            ga_s = p_lst.tile([128, 24], FP32, tag="gas", name=f"gas{t}")
            ga_g = p_lst.tile([128, 8], FP32, tag="gag", name=f"gag{t}")
            # sigmoid via tanh to avoid ACT table swaps mid-stream:
            # sigma(x) = 0.5*tanh(x/2) + 0.5
            nc.scalar.activation(out=ga_s[:, :], in_=xp[:, r, 0:24],
                                 func=AF.Tanh, scale=0.5)
            nc.vector.tensor_scalar(
                out=ga_s[:, :], in0=ga_s[:, :],
                scalar1=0.5, scalar2=0.5, op0=OP.mult, op1=OP.add)
            nc.scalar.activation(out=ga_g[:, :], in_=xp[:, r, 24:32],
                                 func=AF.Tanh)
            tmp = p_lst.tile([128, 8], FP32, tag="tmp", name=f"tmp{t}")
            nc.vector.tensor_tensor(
                out=tmp[:, :], in0=ga_s[:, 0:8], in1=ga_g[:, :], op=OP.mult)
            if t == 0:
                nc.vector.tensor_copy(out=c_sb[:, :], in_=tmp[:, :])
            else:
                nc.vector.tensor_tensor(
                    out=c_sb[:, :], in0=c_sb[:, :], in1=ga_s[:, 8:16],
                    op=OP.mult)
                nc.vector.tensor_tensor(
                    out=c_sb[:, :], in0=c_sb[:, :], in1=tmp[:, :], op=OP.add)
            tch = p_lst.tile([128, 8], FP32, tag="tch", name=f"tch{t}")
            nc.scalar.activation(out=tch[:, :], in_=c_sb[:, :], func=AF.Tanh)
            nc.vector.tensor_tensor(
                out=hsb_sb[:, t, :], in0=ga_s[:, 16:24], in1=tch[:, :],
                op=OP.mult)

        # ================= Phase A (+ woven LSTM) =================
        blocks = [list(range(b, min(b + BLK, n_groups)))
                  for b in range(0, n_groups, BLK)]
        NB = len(blocks)
        embs, ebts, tails, c0s, lgs = {}, {}, {}, {}, {}

        with (
            tc.tile_pool(name="emb", bufs=8) as p_emb,
            tc.tile_pool(name="ebts", bufs=8) as p_ebt,
            tc.tile_pool(name="c0s", bufs=10) as p_c0,
            tc.tile_pool(name="lgs", bufs=8) as p_lg,
            tc.tile_pool(name="small", bufs=8) as p_small,
            tc.tile_pool(name="tpp", bufs=1, space="PSUM") as p_tp,
            tc.tile_pool(name="cvp", bufs=4, space="PSUM") as p_cv,
        ):
            def emit_gathers(bi):
                for g in blocks[bi]:
                    emb_g = p_emb.tile([128, 4, EP], BF16, tag="emb",
                                       name=f"emb{g}")
                    embs[g] = emb_g
                    for s in range(4):
                        nc.gpsimd.indirect_dma_start(
                            out=emb_g[:, s, :],
                            out_offset=None,
                            in_=d_wemb[:, :],
                            in_offset=IndirectOffsetOnAxis(
                                ap=idx_sb[:, 4 * g + s:4 * g + s + 1], axis=0),
                        )

            def emit_transposes(bi):
                for g in blocks[bi]:
                    ebt_g = p_ebt.tile([128, 2, 512], FP8, tag="ebt",
                                       name=f"ebt{g}")
                    tail_g = p_ebt.tile([44, 2, 528], FP8, tag="tail",
                                        name=f"tail{g}")
                    ebts[g], tails[g] = ebt_g, tail_g
                    for ec in range(3):
                        ecw = ECH[ec]
                        tp = p_tp.tile([128, 512], BF16, tag="tp",
                                       name=f"tp{g}_{ec}")
                        for s in range(4):
                            nc.tensor.transpose(
                                out=tp[:ecw, 128 * s:128 * (s + 1)],
                                in_=embs[g][:, s, 128 * ec:128 * ec + ecw],
                                identity=identb[:, :],
                            )
                        if ec == 0:
                            nc.scalar.activation(
                                out=ebt_g[:, ec, :], in_=tp[:, :],
                                func=AF.Identity)
                        elif ec == 1:
                            nc.vector.tensor_copy(
                                out=ebt_g[:, ec, :], in_=tp[:, :])
                        else:
                            nc.vector.tensor_copy(
                                out=tail_g[:, 0, 0:512], in_=tp[:44, 0:512])
                            nc.vector.tensor_copy(
                                out=tail_g[:, 1, 0:511], in_=tp[:44, 1:512])
                            # col 511 of the shifted slot is only hit by the
                            # zero weights; keep it finite so 0*x stays 0
                            nc.vector.memset(tail_g[:, 1, 511:512], 0.0)

            def emit_logit_job(g, mc):
                lp = p_cv.tile([128, 512], FP32, tag="lp", bufs=1,
                               name=f"lp{g}_{mc}")
                nc.tensor.matmul(
                    out=lp[:, 0:508],
                    lhsT=wa0_sb[:, :, mc, :],
                    rhs=c0s[g][:, 0:2, 0:508],
                    start=True, stop=True,
                    perf_mode=DR, skip_group_check=True,
                )
                if g not in lgs:
                    lgs[g] = p_lg.tile([128, 2, 512], BF16, tag="lg",
                                       name=f"lg_{g}")
                nc.scalar.activation(
                    out=lgs[g][:, mc, 0:508], in_=lp[:, 0:508],
                    func=AF.Tanh, bias=ba0_sb[:, mc:mc + 1])

            def emit_conv(bi, weave=(), lgjobs=()):
                poss = blocks[bi]
                cvs = {}
                for fc in range(2):
                    for g in poss:
                        cvs[(fc, g)] = p_cv.tile([128, 512], FP32, tag="cv",
                                                 name=f"cv{fc}_{g}")
                wi = iter(weave)
                li = iter(lgjobs)
                passes = []
                for fc in range(2):
                    for w in range(W):
                        passes.append((fc, 'm', w))
                    for j in range(3):
                        passes.append((fc, 't', j))
                for pi, (fc, kind, w) in enumerate(passes):
                    # weave an LSTM step every ~5 passes
                    if pi in (3, 8, 13):
                        t = next(wi, None)
                        if t is not None:
                            emit_lstm_step(t)
                    elif pi % 2 == 1:
                        job = next(li, None)
                        if job is not None:
                            emit_logit_job(*job)
                    for g in poss:
                        if kind == 'm':
                            nc.tensor.matmul(
                                out=cvs[(fc, g)][:, 0:508],
                                lhsT=wmain_sb[:, w, :, fc, :],
                                rhs=ebts[g][:, 0:2, w:w + 508],
                                start=(w == 0), stop=False,
                                perf_mode=DR, skip_group_check=True,
                            )
                        else:
                            nc.tensor.matmul(
                                out=cvs[(fc, g)][:, 0:508],
                                lhsT=wtail_sb[:, w, :, fc, :],
                                rhs=tails[g][:, 0:2, 2 * w:2 * w + 508],
                                start=False, stop=(w == 2),
                                perf_mode=DR, skip_group_check=True,
                            )
                    # drain each fc's banks as soon as its last pass retires
                    if pi == 7 or pi == 15:
                        fcd = 0 if pi == 7 else 1
                        for g in poss:
                            if g not in c0s:
                                c0s[g] = p_c0.tile([128, 2, 512], FP8,
                                                   tag="c0", name=f"c0_{g}")
                            nc.scalar.activation(
                                out=c0s[g][:, fcd, 0:508],
                                in_=cvs[(fcd, g)][:, 0:508],
                                func=AF.Tanh, bias=bconv_sb[:, fcd:fcd + 1])
                for t in wi:
                    emit_lstm_step(t)
                for job in li:
                    emit_logit_job(*job)

            def emit_softmax(bi):
                poss = blocks[bi]
                for g in poss:
                    ex_g = p_lg.tile([128, 2, 4, 124], BF16, tag="ex",
                                     name=f"ex_{g}")
                    prod = p_lg.tile([128, 2, 4, 124], BF16, tag="prod",
                                     name=f"prod_{g}")
                    den = p_small.tile([128, 2, 4], FP32, tag="den",
                                       name=f"den_{g}")
                    num = p_small.tile([128, 2, 4], FP32, tag="num",
                                       name=f"num_{g}")
                    lgv = lgs[g].rearrange("p m (s t) -> p m s t", s=4)
                    c0v = c0s[g].rearrange("p m (s t) -> p m s t", s=4)
                    for mc in range(2):
                        nc.scalar.activation(
                            out=ex_g[:, mc, :, :],
                            in_=lgv[:, mc, :, 0:T],
                            func=AF.Exp)
                    nc.vector.tensor_reduce(
                        out=den[:, :, :], in_=ex_g[:, :, :, :],
                        axis=AX.X, op=OP.add)
                    nc.vector.tensor_tensor(
                        out=prod[:, :, :, :], in0=ex_g[:, :, :, :],
                        in1=c0v[:, :, :, 0:T], op=OP.mult)
                    nc.vector.tensor_reduce(
                        out=num[:, :, :], in_=prod[:, :, :, :],
                        axis=AX.X, op=OP.add)
                    nc.vector.reciprocal(out=den[:, :, :], in_=den[:, :, :])
                    # ---- s0 = num/den scattered to (t, d) slots ----
                    if uniform_pm:
                        nc.vector.tensor_tensor(
                            out=s0T_sb[:, :, g, :],
                            in0=num[:, :, :], in1=den[:, :, :], op=OP.mult)
                    else:
                        tgt = group_targets[g]
                        for mc in range(2):
                            for (s_lo, n_run, td) in _target_runs(tgt):
                                if td is None:
                                    out_ap = s0scr[:, mc, s_lo:s_lo + n_run]
                                else:
                                    t0, d0 = td
                                    out_ap = s0T_sb[:, mc, t0:t0 + n_run,
                                                    d0:d0 + 1]
                                nc.vector.tensor_tensor(
                                    out=out_ap,
                                    in0=num[:, mc, s_lo:s_lo + n_run],
                                    in1=den[:, mc, s_lo:s_lo + n_run],
                                    op=OP.mult)

            # -------- emission schedule --------
            if uniform_pm:
                emit_gathers(0)
                if NB > 1:
                    emit_gathers(1)
                emit_transposes(0)
                next_step = 0
                chunks_done = 0
                for b in range(NB):
                    if b + 2 < NB:
                        emit_gathers(b + 2)
                    if b + 1 < NB:
                        emit_transposes(b + 1)
                    # weave LSTM steps gated on emitted xproj chunks
                    avail = 8 * chunks_done
                    weave = []
                    import os as _os
                    if not _os.environ.get("NOWEAVE"):
                        while (next_step < min(avail, S_eff)
                               and len(weave) < BLK):
                            weave.append(next_step)
                            next_step += 1
                    lgjobs = ([(g, mc) for g in blocks[b - 1]
                               for mc in range(2)] if b > 0 else [])
                    emit_conv(b, weave=weave, lgjobs=lgjobs)
                    if b > 0:
                        emit_softmax(b - 1)
                    if dbg and b == 1:
                        nc.sync.dma_start(out=d_dbg_c0[:, :, :],
                                          in_=c0s[0][:, :, :])
                    done = min(b * BLK, n_groups)  # softmax done thru b-1
                    while (chunks_done < n_chunks
                           and 8 * (chunks_done + 1) <= done):
                        emit_xproj_chunk(chunks_done)
                        chunks_done += 1
                for g in blocks[NB - 1]:
                    for mc in range(2):
                        emit_logit_job(g, mc)
                emit_softmax(NB - 1)
                while chunks_done < n_chunks:
                    emit_xproj_chunk(chunks_done)
                    chunks_done += 1
                rem_start = next_step
            else:
                for b in range(NB):
                    if b == 0:
                        emit_gathers(0)
                        if NB > 1:
                            emit_gathers(1)
                        emit_transposes(0)
                    if b + 2 < NB:
                        emit_gathers(b + 2)
                    if b + 1 < NB:
                        emit_transposes(b + 1)
                    emit_conv(b)
                    for g in blocks[b]:
                        for mc in range(2):
                            emit_logit_job(g, mc)
                    emit_softmax(b)
                for c in range(n_chunks):
                    emit_xproj_chunk(c)
                rem_start = 0

        if dbg:
            nc.sync.dma_start(out=d_dbg_s0[:, :, :, :],
                              in_=s0T_sb[:, :, :, :])
            xpcp = consts.tile([128, 2, 16, 32], FP32)
            for i in range(2):
                nc.vector.tensor_copy(out=xpcp[:, i, :, :],
                                      in_=xps[i][:, :, :])
            nc.sync.dma_start(out=d_dbg_xp[:, :, :, :],
                              in_=xpcp[:, :, :, :])

        # ================= remaining LSTM steps =================
        for t in range(rem_start, S_eff):
            emit_lstm_step(t)
        if dbg:
            nc.sync.dma_start(out=d_dbg_h[:, :, :], in_=hsb_sb[:, :, :])

        # ================= Phase D: sentence attention + output ============
        with (
            tc.tile_pool(name="a1p", bufs=2, space="PSUM") as p_a1p,
            tc.tile_pool(name="a1s", bufs=2) as p_a1s,
        ):
            l1_sb = p_a1s.tile([128, 2, S4], FP32, tag="l1")
            for mc in range(2):
                l1_ps = p_a1p.tile([128, S4], FP32, tag="l1p")
                for kc in range(2):
                    nc.tensor.matmul(
                        out=l1_ps[:, :],
                        lhsT=wa1_sb[:, kc, mc, :],
                        rhs=hsb_sb[:, :, 4 * kc:4 * kc + 4],
                        start=(kc == 0), stop=(kc == 1),
                    )
                nc.scalar.activation(
                    out=l1_sb[:, mc, :], in_=l1_ps[:, :],
                    func=AF.Tanh, bias=ba1_sb[:, mc:mc + 1])
                nc.vector.tensor_tensor(
                    out=l1_sb[:, mc, :], in0=l1_sb[:, mc, :],
                    in1=mb1_sb[:, :], op=OP.add)
            # softmax over t per (mc, d) with segmented reduces
            ex1 = p_a1s.tile([128, 2, DPC, S_eff], FP32, tag="ex1")
            pr1 = p_a1s.tile([128, 2, DPC, S_eff], FP32, tag="pr1")
            den1 = p_a1s.tile([128, 2, DPC], FP32, tag="den1")
            num1 = p_a1s.tile([128, 2, DPC], FP32, tag="num1")
            l1v = l1_sb.rearrange("p m (t d) -> p m t d", d=DPC)
            ex1v = ex1.rearrange("p m d t -> p m t d")
            for mc in range(2):
                nc.scalar.activation(
                    out=ex1v[:, mc, :, :], in_=l1v[:, mc, :, :],
                    func=AF.Exp)
            nc.vector.tensor_reduce(
                out=den1[:, :, :], in_=ex1[:, :, :, :], axis=AX.X, op=OP.add)
            hsv = hsb_sb.rearrange("p t (k d) -> p k d t", k=2)
            pr1v = pr1[:, :, :, :]
            nc.vector.tensor_tensor(
                out=pr1v, in0=ex1[:, :, :, :], in1=hsv[:, :, :, :],
                op=OP.mult)
            nc.vector.tensor_reduce(
                out=num1[:, :, :], in_=pr1[:, :, :, :], axis=AX.X, op=OP.add)
            nc.vector.reciprocal(out=den1[:, :, :], in_=den1[:, :, :])
            s1_sb = p_a1s.tile([128, 2, DPC], BF16, tag="s1")
            nc.vector.tensor_tensor(
                out=s1_sb[:, :, :], in0=num1[:, :, :], in1=den1[:, :, :],
                op=OP.mult)
            o_ps = p_a1p.tile([128, DPC], FP32, tag="op")
            for kc in range(2):
                nc.tensor.matmul(
                    out=o_ps[:1, :],
                    lhsT=wo_sb[:, kc:kc + 1],
                    rhs=s1_sb[:, kc, :],
                    start=(kc == 0), stop=(kc == 1),
                )
            y_sb = p_a1s.tile([1, DPC], FP32, tag="y")
            nc.scalar.activation(
                out=y_sb[:, :], in_=o_ps[:1, :],
                func=AF.Tanh, bias=boh_sb[:1, :1], scale=0.5)
            nc.vector.tensor_scalar(
                out=y_sb[:, :], in0=y_sb[:, :],
                scalar1=0.5, scalar2=0.5, op0=OP.mult, op1=OP.add)
            nc.sync.dma_start(out=d_out[:, :], in_=y_sb[:, :])

    return nc


def _target_runs(tgt):
    """Compress 4 per-sentence (t, d)/None targets into (start, len, td) runs
    where a run covers consecutive t at fixed d (or None-pads)."""
    runs = []
    i = 0
    while i < 4:
        if tgt[i] is None:
            j = i
            while j < 4 and tgt[j] is None:
                j += 1
            runs.append((i, j - i, None))
            i = j
        else:
            t0, d0 = tgt[i]
            j = i + 1
            while j < 4 and tgt[j] is not None and tgt[j] == (t0 + (j - i), d0):
                j += 1
            runs.append((i, j - i, (t0, d0)))
            i = j
    return runs


def _host_prep(inputs):
    inp = {k: np.asarray(v) for k, v in inputs.items()}
    tok = inp["input"].astype(np.int32)
    num_sent = inp["num_sent"].astype(np.int64)
    mask = np.asarray(inp["mask"], np.float32)

    S_eff = max(int(num_sent.max()), 1)
    # ragged mapping exactly like the reference scatter
    batch_ids = np.repeat(np.arange(B), num_sent)
    if len(batch_ids) < TOTAL:
        batch_ids = np.concatenate(
            [batch_ids, np.full(TOTAL - len(batch_ids), B - 1, np.int64)])
    batch_ids = batch_ids[:TOTAL]
    offsets = np.cumsum(num_sent) - num_sent
    pos = np.arange(TOTAL) - offsets[batch_ids]
    valid = pos < num_sent[batch_ids]

    per_core = []
    for k in range(NCORES):
        sids = np.where((batch_ids // DPC == k) & valid)[0]
        ents = [(int(j), int(batch_ids[j] % DPC), int(pos[j])) for j in sids]
        # position-major: all docs' sentence t adjacent -> group g == slot t
        ents.sort(key=lambda e: (e[2], e[1]))
        per_core.append(ents)
    n_groups = max(1, (max(len(pc) for pc in per_core) + 3) // 4)
    NLOC = n_groups * 4

    # per-core group target maps; must agree across cores for the shared
    # program (true for uniform num_sent). Fall back handled by caller.
    tmaps = []
    for k in range(NCORES):
        tm = []
        for j in range(NLOC):
            if j < len(per_core[k]):
                _, d, p = per_core[k][j]
                tm.append((p, d))
            else:
                tm.append(None)
        tmaps.append(tm)
    uniform = all(tm == tmaps[0] for tm in tmaps)

    group_targets = [tmaps[0][4 * g:4 * g + 4] for g in range(n_groups)]

    in_maps = []
    wembt = np.zeros((V, EP), bf16)
    wembt[:, 0:E] = np.asarray(inp["Wemb"], np.float32).astype(bf16)
    wc = np.asarray(inp["Wconv"], np.float32)  # [F,1,W,E]
    # main conv weights: [p, w, pair, fc, m] = Wconv[fc*128+m, 0, w, pair*128+p]
    wmain = np.zeros((128, W, 2, 2, 128), f8)
    for w in range(W):
        for pr in range(2):
            for fc in range(2):
                blk = wc[128 * fc:128 * (fc + 1), 0, w,
                         128 * pr:128 * (pr + 1)]
                wmain[:, w, pr, fc, :] = blk.T.astype(f8)
    # tail conv weights: pairs (w0,w1), (w2,w3), (w4, zero) over E rows 256:300
    wtail = np.zeros((44, 3, 2, 2, 128), f8)
    for j in range(3):
        for i in range(2):
            w = 2 * j + i
            if w >= W:
                continue
            for fc in range(2):
                blk = wc[128 * fc:128 * (fc + 1), 0, w, 256:300]
                wtail[:, j, i, fc, :] = blk.T.astype(f8)
    bconv_t = np.asarray(inp["bconv"], np.float32).reshape(2, 128).T.copy()
    wa0_t = _pack_kx(inp["Wa0"], f8)
    ba0_t = np.asarray(inp["ba0"], np.float32).reshape(2, 128).T.copy()
    # LSTM weights with gate-chunk order (i,i,f,f,o,o,g,g)
    wih_t = _pack_kx(np.asarray(inp["Wih"], np.float32).T, bf16)[:, :, GPERM, :]
    whh_t = _pack_kx(np.asarray(inp["Whh"], np.float32).T, bf16)[:, :, GPERM, :]
    bihrow = np.ascontiguousarray(
        (np.asarray(inp["bih"], np.float32)
         + np.asarray(inp["bhh"], np.float32)).reshape(8, 128)[GPERM][None]
    ).astype(bf16)
    wa1_t = _pack_kx(inp["Wa1"], bf16)
    ba1_t = np.asarray(inp["ba1"], np.float32).reshape(2, 128).T.copy()
    wo_t = np.asarray(inp["Wo"], np.float32).reshape(2, 128).T.astype(bf16).copy()
    bo_half = (0.5 * np.asarray(inp["bo"], np.float32)).reshape(1, 1)

    for k in range(NCORES):
        idx_t = np.zeros((128, NLOC), np.int32)
        for j, (sj, _, _) in enumerate(per_core[k]):
            idx_t[:, j] = tok[sj]
        mb1 = np.zeros((1, S_eff * DPC), np.float32)
        for d in range(DPC):
            doc = k * DPC + d
            mvals = mask[doc, :S_eff, 0]
            mb1[0, np.arange(S_eff) * DPC + d] = np.where(mvals > 0, 0.0, -1e9)
        in_maps.append({
            "idx_t": idx_t, "wembt": wembt, "wmain": wmain, "wtail": wtail,
            "bconv_t": bconv_t, "wa0_t": wa0_t, "ba0_t": ba0_t,
            "wih_t": wih_t, "bihrow": bihrow, "whh_t": whh_t,
            "wa1_t": wa1_t, "ba1_t": ba1_t, "mb1": mb1,
            "wo_t": wo_t, "bo_half": bo_half,
        })
    return S_eff, n_groups, group_targets, tmaps, uniform, in_maps


def _pack_kx(w, dt):
    """[K=256, M_total] -> [128, kc, mt, 128] tile pack."""
    w = np.asarray(w, np.float32)
    K, M = w.shape
    assert K == 256 and M % 128 == 0
    mt = M // 128
    out = np.zeros((128, 2, mt, 128), dt)
    for kc in range(2):
        for m in range(mt):
            out[:, kc, m, :] = w[128 * kc:128 * (kc + 1),
                                 128 * m:128 * (m + 1)].astype(dt)
    return out


_NC_CACHE = {}


def kernel(**inputs) -> np.ndarray:
    S_eff, n_groups, group_targets, tmaps, uniform, in_maps = _host_prep(inputs)

    out = np.zeros((B, 1), np.float32)
    if uniform:
        key = (S_eff, n_groups, tuple(tuple(t) if t else None
                                      for g in group_targets for t in g))
        if key not in _NC_CACHE:
            _NC_CACHE[key] = build_nc(S_eff, n_groups, group_targets)
        nc = _NC_CACHE[key]
        res = run_bass_kernel_spmd(nc, in_maps, core_ids=list(range(NCORES)))
        for k in range(NCORES):
            out[k * DPC:(k + 1) * DPC, 0] = res.results[k]["out"][0]
    else:
        # ragged fallback: per-core programs
        from concourse.bass_utils import run_bass_kernel
        for k in range(NCORES):
            gt_k = [tmaps[k][4 * g:4 * g + 4] for g in range(n_groups)]
            nc = build_nc(S_eff, n_groups, gt_k)
            r = run_bass_kernel(nc, in_maps[k], core_id=0)
            out[k * DPC:(k + 1) * DPC, 0] = r["out"][0]
    return out
